# revision 14
# baseline (speedup 1.0000x reference)
"""Trainium2 Bass kernel for nn_Attention_111669150032.

Multi-head attention (B=2, S=2048, DIM=1024, NH=16, HD=64) with RoPE,
returning (x @ wq/wk/wv -> attention -> @ wo, attn_weights).

Sharding: tensor-parallel over heads across 8 cores (2 heads/core).
wq/wk/wv sharded on output features (column-parallel), wo row-parallel;
partial outputs summed on host. attn_weights written per-core (its 2
heads) and concatenated on host.

Per-core pipeline (all matmuls bf16, fp32 PSUM accumulation):
  1. QKV projections: lhsT = W^T tiles (stationary), rhs = x^T (shipped
     pre-transposed from host) -> Q^T/K^T/V^T [feat, t] layouts.
  2. RoPE on Q^T/K^T in [d, t] layout. Host permutes wq/wk rows so the
     head dim is deinterleaved ([evens, odds]); a permutation matmul
     (PI) produces the half-swapped copy, then
     Q'^T = Q^T*CC + (PI Q^T)*SS   (CC/SS precomputed cos/sin maps with
     signs baked in). Scores are invariant to the shared permutation.
  3. V^T -> V via PE transpose (V is the AV-matmul stationary operand).
  4. Scores S = Q'^T.T @ K'^T per head, two heads packed into the PE
     array via row tiling (K=64 each).
  5. exp via ScalarE activation reading PSUM, scale=1/8 folded in,
     accum_out gives softmax row sums for free. P = E * (1/Z) via
     per-partition tensor_scalar.
  6. P^T via PE transposes (bf16) for the AV matmul; AV col-packed
     (2 heads, M=64 each) accumulating U^T = V^T P^T over k blocks.
  7. Out-projection out = U @ woS per 128-token chunk, partials to DRAM.
"""

import numpy as np
import ml_dtypes

B, S, DIM, NH = 2, 2048, 1024, 16
HD = DIM // NH           # 64
NCORES = 8
NHL = NH // NCORES       # 2 heads per core
F = NHL * HD             # 128 features per core
ROPE_THETA = 500000.0

_BF = ml_dtypes.bfloat16


def build_program(s=S):
    import concourse.bass as bass
    import concourse.mybir as mybir
    import concourse.tile as tile
    from concourse import bacc

    fp32 = mybir.dt.float32
    bf16 = mybir.dt.bfloat16
    t_total = B * s

    nt512 = t_total // 512      # token chunks of 512 for projections
    nqi = s // 128              # q chunks of 128 per batch
    kh = s // 2                 # exp free-dim (half a score row)
    nkc = max(1, kh // 512)     # matmul N-chunks per half
    kn = min(512, kh)           # scores matmul N
    qg = min(512, s)            # q group for AV / rhs N
    nqj = s // qg
    qpg = qg // 128
    nkb = s // 128              # k blocks of 128

    nc = bacc.Bacc("TRN2")
    # all small inputs packed into one [128, cbw] blob (single DMA -> one
    # sem lane; scattered input DMAs blow the per-instruction sync-wait cap)
    cbw = 8 * 3 * F + DIM + 128 + F + 2 * t_total
    cblob = nc.dram_tensor("cblob", [128, cbw], bf16, kind="ExternalInput")
    xTb = nc.dram_tensor("xTb", [128, 8 * t_total], bf16, kind="ExternalInput")
    attnP = nc.dram_tensor("attnP", [B, NHL, s, s], bf16, kind="ExternalOutput")
    outP = nc.dram_tensor("outP", [t_total, DIM], bf16, kind="ExternalOutput")

    Exp = mybir.ActivationFunctionType.Exp

    with tile.TileContext(nc) as tc:
        with tc.tile_pool(name="const", bufs=1) as constp:
            cb_sb = constp.tile([128, cbw], bf16, name="cb_sb")
            nc.sync.dma_start(cb_sb[:], cblob[:])
            o = 0
            wqkv_sb = cb_sb[:, o:o + 8 * 3 * F].rearrange(
                "p (i f) -> p i f", i=8)
            o += 8 * 3 * F
            woS_sb = cb_sb[:, o:o + DIM]
            o += DIM
            idm_sb = cb_sb[:, o:o + 128]
            o += 128
            pim_sb = cb_sb[:, o:o + F]
            o += F
            cc_sb = cb_sb[:, o:o + t_total]
            o += t_total
            ss_sb = cb_sb[:, o:o + t_total]

            qpt = constp.tile([F, t_total], bf16, name="qpt")
            kpt = constp.tile([F, t_total], bf16, name="kpt")
            vr = constp.tile([128, t_total // 128, 128], bf16, name="vr")
            ut = constp.tile([128, t_total], bf16, name="ut")

            # ---- phase 1: QKV projections + RoPE + V transpose ----
            with tc.tile_pool(name="proj", bufs=1) as projp:
                xT_big = projp.tile([128, 8 * t_total], bf16, name="xT_big")
                nc.sync.dma_start(xT_big[:], xTb[:])
                xT_sb = xT_big.rearrange("p (i t) -> p i t", i=8)

                with (
                    tc.tile_pool(name="pps", space="PSUM", bufs=2) as pps,
                    tc.tile_pool(name="ptmp", bufs=3) as ptmp,
                ):
                    for p in range(3):
                        for j in range(nt512):
                            ps = pps.tile([128, 512], fp32, tag="proj_ps",
                                          bufs=2, name="ps")
                            for i in range(8):
                                nc.tensor.matmul(
                                    ps[:],
                                    lhsT=wqkv_sb[:, i, F * p:F * (p + 1)],
                                    rhs=xT_sb[:, i, 512 * j:512 * (j + 1)],
                                    start=(i == 0), stop=(i == 7),
                                )
                            if p < 2:
                                dst = qpt if p == 0 else kpt
                                pre = ptmp.tile([128, 512], bf16, tag="pre",
                                                bufs=3, name="pre")
                                nc.scalar.copy(pre[:], ps[:])
                                rot = pps.tile([128, 512], fp32, tag="rot_ps",
                                               bufs=2, name="rot")
                                nc.tensor.matmul(rot[:], lhsT=pim_sb[:],
                                                 rhs=pre[:], start=True,
                                                 stop=True)
                                t1 = ptmp.tile([128, 512], bf16, tag="t1",
                                               bufs=3, name="t1")
                                nc.vector.tensor_mul(
                                    t1[:], pre[:], cc_sb[:, 512 * j:512 * (j + 1)])
                                t2 = ptmp.tile([128, 512], bf16, tag="t2",
                                               bufs=3, name="t2")
                                nc.vector.tensor_mul(
                                    t2[:], rot[:], ss_sb[:, 512 * j:512 * (j + 1)])
                                nc.vector.tensor_add(
                                    dst[:, 512 * j:512 * (j + 1)], t1[:], t2[:])
                            else:
                                vt = ptmp.tile([128, 512], bf16, tag="vt",
                                               bufs=3, name="vt")
                                nc.scalar.copy(vt[:], ps[:])
                                tp = pps.tile([128, 512], bf16, tag="tp_ps",
                                              bufs=2, name="tp")
                                for u in range(4):
                                    nc.tensor.transpose(
                                        tp[:, 128 * u:128 * (u + 1)],
                                        vt[:, 128 * u:128 * (u + 1)],
                                        idm_sb[:])
                                nc.vector.tensor_copy(
                                    vr[:, 4 * j:4 * (j + 1), :], tp[:])

            # ---- phase 2: attention ----
            with (
                tc.tile_pool(name="aps", space="PSUM", bufs=1) as aps,
                tc.tile_pool(name="epool", bufs=5) as epool,
                tc.tile_pool(name="ppool", bufs=3) as ppool,
                tc.tile_pool(name="etpool", bufs=2) as etpool,
                tc.tile_pool(name="opool", bufs=3) as opool,
            ):
                for b in range(B):
                    for qj in range(nqj):
                        et = {}
                        zc = {}
                        zrq = {}
                        for h in (0, 1):
                            et[h] = etpool.tile([128, nkb, qg], bf16,
                                                tag=f"et{h}", bufs=2,
                                                name=f"et{h}")
                            zc[h] = epool.tile([128, 2 * qpg], fp32,
                                               tag=f"zc{h}", bufs=3,
                                               name=f"zc{h}")
                            zrq[h] = epool.tile([128, qpg], fp32,
                                                tag=f"zrq{h}", bufs=3,
                                                name=f"zrq{h}")
                        e_store = {}
                        for u in range(qpg):
                            qi = qj * qpg + u
                            q0 = b * s + 128 * qi
                            sps = {}
                            for half in (0, 1):
                                for kc in range(nkc):
                                    for h in (0, 1):
                                        if kc == 0 and half == 0:
                                            sps[h] = aps.tile(
                                                [128, kh], fp32, tag=f"s{h}",
                                                bufs=1, name=f"s{h}")
                                            e_store[(u, h)] = epool.tile(
                                                [128, s], bf16, tag=f"e{h}",
                                                bufs=qpg + 1, name=f"e{h}")
                                        k0 = b * s + half * kh + kn * kc
                                        nc.tensor.matmul(
                                            sps[h][:, kn * kc:kn * (kc + 1)],
                                            lhsT=qpt[64 * h:64 * (h + 1),
                                                     q0:q0 + 128],
                                            rhs=kpt[64 * h:64 * (h + 1),
                                                    k0:k0 + kn],
                                            start=True, stop=True,
                                            tile_position=(64 * h, 0),
                                        )
                                for h in (0, 1):
                                    nc.scalar.activation(
                                        e_store[(u, h)][:, half * kh:
                                                        (half + 1) * kh],
                                        sps[h][:],
                                        Exp, scale=0.125,
                                        accum_out=zc[h][:, 2 * u + half:
                                                        2 * u + half + 1],
                                    )
                        # row sums -> reciprocal (per qi: only 2 producers)
                        for h in (0, 1):
                            zsq = epool.tile([128, qpg], fp32, tag=f"zsq{h}",
                                             bufs=3, name=f"zsq{h}")
                            for u in range(qpg):
                                nc.vector.tensor_add(
                                    zsq[:, u:u + 1], zc[h][:, 2 * u:2 * u + 1],
                                    zc[h][:, 2 * u + 1:2 * u + 2])
                            nc.vector.reciprocal(zrq[h][:], zsq[:])
                        # normalize + store + transpose (after zr available)
                        for u in range(qpg):
                            qi = qj * qpg + u
                            for h in (0, 1):
                                p_sb = ppool.tile([128, s], bf16, tag=f"p{h}",
                                                  bufs=3, name=f"p{h}")
                                nc.vector.tensor_scalar_mul(
                                    p_sb[:], e_store[(u, h)][:],
                                    zrq[h][:, u:u + 1])
                                nc.sync.dma_start(
                                    attnP[b, h, 128 * qi:128 * (qi + 1), :],
                                    p_sb[:])
                                tps = aps.tile([128, nkb * 128], bf16,
                                               tag="ets", bufs=1, name="tps")
                                for kb in range(nkb):
                                    nc.tensor.transpose(
                                        tps[:, 128 * kb:128 * (kb + 1)],
                                        p_sb[:, 128 * kb:128 * (kb + 1)],
                                        idm_sb[:])
                                nc.vector.tensor_copy(
                                    et[h][:, :, 128 * u:128 * (u + 1)],
                                    tps.rearrange("p (k c) -> p k c", c=128))
                        # AV for the whole q group
                        ups = aps.tile([128, qg], fp32, tag="u", bufs=1,
                                       name="ups")
                        for kb in range(nkb):
                            for h in (0, 1):
                                nc.tensor.matmul(
                                    ups[64 * h:64 * (h + 1), :],
                                    lhsT=vr[:, b * nkb + kb,
                                            64 * h:64 * (h + 1)],
                                    rhs=et[h][:, kb, :],
                                    start=(kb == 0), stop=(kb == nkb - 1),
                                    tile_position=(0, 64 * h),
                                    skip_group_check=True,
                                )
                        nc.scalar.copy(ut[:, b * s + qg * qj:
                                          b * s + qg * (qj + 1)], ups[:])
                        # out projection
                        for u2 in range(qpg):
                            q0 = b * s + qg * qj + 128 * u2
                            ost = opool.tile([128, DIM], bf16, tag="ost",
                                             bufs=3, name="ost")
                            for oc in range(2):
                                ops_ = aps.tile([128, 512], fp32, tag="o",
                                                bufs=1, name="ops")
                                nc.tensor.matmul(
                                    ops_[:],
                                    lhsT=ut[:, q0:q0 + 128],
                                    rhs=woS_sb[:, 512 * oc:512 * (oc + 1)],
                                    start=True, stop=True)
                                nc.vector.tensor_copy(
                                    ost[:, 512 * oc:512 * (oc + 1)], ops_[:])
                            nc.sync.dma_start(outP[q0:q0 + 128, :], ost[:])
    nc.finalize()
    return nc


def host_inputs(x, wq, wk, wv, wo, freqs_cos, freqs_sin, s=S):
    """Build per-core input maps (numpy, bf16)."""
    t_total = B * s
    x2 = np.asarray(x, dtype=np.float32)[:, :s].reshape(t_total, DIM)
    xT = np.ascontiguousarray(x2.T).astype(_BF)

    # deinterleave permutation within each head (evens then odds)
    perm = np.empty(DIM, dtype=np.int64)
    for h in range(NH):
        base = h * HD
        perm[base:base + HD // 2] = base + 2 * np.arange(HD // 2)
        perm[base + HD // 2:base + HD] = base + 2 * np.arange(HD // 2) + 1
    wq_p = np.asarray(wq, np.float32)[perm]
    wk_p = np.asarray(wk, np.float32)[perm]
    wv_ = np.asarray(wv, np.float32)
    wo_ = np.asarray(wo, np.float32)

    cos = np.asarray(freqs_cos, np.float32)[:s]  # [s, 32]
    sin = np.asarray(freqs_sin, np.float32)[:s]
    cc = np.empty((F, t_total), np.float32)
    ssn = np.empty((F, t_total), np.float32)
    cosT = cos.T  # [32, s]
    sinT = sin.T
    for hl in range(NHL):
        for bb in range(B):
            sl = slice(bb * s, (bb + 1) * s)
            cc[64 * hl:64 * hl + 32, sl] = cosT
            cc[64 * hl + 32:64 * hl + 64, sl] = cosT
            ssn[64 * hl:64 * hl + 32, sl] = -sinT
            ssn[64 * hl + 32:64 * hl + 64, sl] = sinT
    pim = np.zeros((F, F), np.float32)
    for hl in range(NHL):
        for j in range(32):
            pim[64 * hl + j, 64 * hl + 32 + j] = 1.0
            pim[64 * hl + 32 + j, 64 * hl + j] = 1.0
    idm = np.eye(128, dtype=np.float32)

    # xTb: [128, 8*t] with free = (d_block, t)
    xTb = np.ascontiguousarray(
        xT.reshape(8, 128, t_total).transpose(1, 0, 2).reshape(
            128, 8 * t_total))

    in_maps = []
    for c in range(NCORES):
        rows = slice(F * c, F * (c + 1))
        wqkvT = np.concatenate(
            [wq_p[rows].T, wk_p[rows].T, wv_[rows].T], axis=1)  # [DIM, 384]
        # [128, 8, 384]: partition = d%128, then d_block
        wqkv_b = wqkvT.reshape(8, 128, 3 * F).transpose(1, 0, 2).reshape(
            128, 8 * 3 * F)
        woS = wo_[:, rows].T  # [F, DIM]
        blob = np.concatenate(
            [wqkv_b, woS, idm, pim, cc, ssn], axis=1).astype(_BF)
        m = {"cblob": np.ascontiguousarray(blob), "xTb": xTb}
        in_maps.append(m)
    return in_maps


def assemble_outputs(results, s=S):
    """results: list of per-core dicts with attnP [B,NHL,s,s] bf16 and
    outP [B*s, DIM] bf16."""
    attn = np.empty((B, NH, s, s), np.float32)
    out = np.zeros((B * s, DIM), np.float32)
    for c, r in enumerate(results):
        attn[:, NHL * c:NHL * (c + 1)] = np.asarray(r["attnP"]).astype(
            np.float32)
        out += np.asarray(r["outP"]).astype(np.float32)
    return out.reshape(B, s, DIM), attn


_CACHE = {}


def kernel(x, wq, wk, wv, wo, freqs_cos, freqs_sin):
    from concourse import bass_utils

    if "nc" not in _CACHE:
        _CACHE["nc"] = build_program(S)
    nc = _CACHE["nc"]
    in_maps = host_inputs(x, wq, wk, wv, wo, freqs_cos, freqs_sin, S)
    res = bass_utils.run_bass_kernel_spmd(
        nc, in_maps, core_ids=list(range(NCORES)))
    out, attn = assemble_outputs(res.results, S)
    return out, attn
